# revision 28
# baseline (speedup 1.0000x reference)
"""Causal single-head attention (B=4, N=4096, d_in=1024, d_inner=512, d_out=1024)
for 8 Trainium2 NeuronCores.

Sharding: core c -> (batch b=c//2, half h=c%2). Core h of a pair owns the 4
global 512-row blocks {2t+h : t=0..3} of batch b, which serve BOTH as its
query blocks and as the key strips whose K/V projection it computes. The
missing (peer-parity) K/V strips arrive via 4 pipelined pair-wise AllGather
collectives (~1MB contribution each) that overlap with projection compute,
eliminating the duplicated K/V projection of the all-local scheme.

Key-slot layout after the gather is rank-major: kt slot s = r*4 + t holds
global strip 2t+r. Attention per query super-block u processes slots
{0..u} u {4..4+u}; causal masking enters only through a per-core 0/1 mask
input so the instruction stream stays SPMD-uniform. The rank1 diagonal
strip's blocks are truncated via free-dim offsets (queries below the
diagonal are never computed, read, or accumulated).

All matmuls run in fp16 (full PE rate, ~1e-3 rel err overall); PSUM
accumulation is fp32. The softmax denominator is accumulated on the Pool
engine (P_acc += P per block) with a single ones-matmul per super-block,
keeping the PE free of the per-block reduction matmuls.
"""

import sys

if "/opt/trn_rl_repo" not in sys.path:
    sys.path.insert(0, "/opt/trn_rl_repo")

import numpy as np

import concourse.bacc as bacc
import concourse.mybir as mybir
import concourse.tile as tile
from concourse.bass_utils import run_bass_kernel_spmd

P = 128
B, N, DIN, DI, DO = 4, 4096, 1024, 512, 1024
NCORES = 8
NQ = N // 2          # query rows per core (2048)
NT = 4               # rounds / query super-blocks per core
SCALE = float(DI) ** -0.5

F32 = mybir.dt.float32
F32R = mybir.dt.float32r
FP16 = mybir.dt.float16
AF = mybir.ActivationFunctionType

GROUPS = [[0, 1], [2, 3], [4, 5], [6, 7]]

_COMPILED = None


def _build():
    nc = bacc.Bacc(None, target_bir_lowering=False)

    xt_d = nc.dram_tensor("xt", [DIN, NQ], FP16, kind="ExternalInput")
    wq_d = nc.dram_tensor("wq", [DIN, DI], FP16, kind="ExternalInput")
    wk_d = nc.dram_tensor("wk", [DIN, DI], FP16, kind="ExternalInput")
    wv_d = nc.dram_tensor("wv", [DIN, DI], FP16, kind="ExternalInput")
    wout_d = nc.dram_tensor("wout", [DI, DO], FP16, kind="ExternalInput")
    bout_d = nc.dram_tensor("bout", [P, DO], FP16, kind="ExternalInput")
    mask_d = nc.dram_tensor("mask", [P, 8, 512], FP16, kind="ExternalInput")
    y_d = nc.dram_tensor("y", [NQ, DO], FP16, kind="ExternalOutput")

    with tile.TileContext(nc) as tc:
        with tc.tile_pool(name="persist", bufs=1) as pp, tc.tile_pool(
            name="dram", bufs=1, space="DRAM"
        ) as dram:
            kt = pp.tile([P, 4, 8, 512], FP16)     # K^T [dk-chunk, slot, j]
            vt = pp.tile([P, 32, DI], FP16)        # V [kbslot, dv]
            qts = [pp.tile([P, 4, 512], FP16, name=f"qt{u}") for u in range(NT)]
            wout = pp.tile([P, 4, DO], FP16)
            bout = pp.tile([P, DO], FP16)
            mask = pp.tile([P, 8, 512], FP16)
            ones = pp.tile([P, P], FP16)
            ones_r = pp.tile([P, P], F32R)
            paccs = [pp.tile([P, 512], F32R, name=f"pacc{i}", tag=f"pacc{i}",
                             bufs=1) for i in range(2)]
            # phase-B working tiles live in the persistent pool so the first
            # attention block never waits on phase-A pool teardown
            pts = [pp.tile([P, 512], FP16, name=f"pt{i}", tag=f"pt{i}", bufs=1)
                   for i in range(4)]
            recips = [pp.tile([P, 512], F32, name=f"rec{i}", tag=f"rec{i}", bufs=1)
                      for i in range(2)]
            attns = [pp.tile([P, 512], FP16, name=f"at{i}", tag=f"at{i}", bufs=1)
                     for i in range(8)]
            y_ss = [pp.tile([P, DO], FP16, name=f"ysb{i}", tag=f"ysb{i}", bufs=1)
                    for i in range(4)]

            cc_ins = [dram.tile([P, 4096], FP16, name=f"ccin{t}") for t in range(NT)]
            cc_outs = [
                dram.tile([2, P, 4096], FP16, name=f"ccout{t}") for t in range(NT)
            ]
            warm_in = dram.tile([P, 4], FP16)
            warm_out = dram.tile([2, P, 4], FP16)

            # ---- Phase A: project own-strip K/V/Q; pair-gather K/V ----
            pa = tc.tile_pool(name="phaseA", bufs=1)
            wp = pa.__enter__()
            psA_cm = tc.tile_pool(name="psA", bufs=1, space="PSUM")
            psA = psA_cm.__enter__()

            wk = wp.tile([P, 8, DI], FP16)
            wv = wp.tile([P, 8, DI], FP16)
            wq = wp.tile([P, 8, DI], FP16)

            wk_src = wk_d.ap().rearrange("(a p) n -> p a n", p=P)
            wv_src = wv_d.ap().rearrange("(a p) n -> p a n", p=P)
            for c in range(8):
                nc.gpsimd.dma_start(wk[:, c, :], wk_src[:, c, :])
            for c in range(8):
                nc.gpsimd.dma_start(wv[:, c, :], wv_src[:, c, :])
            nc.vector.memset(ones[:], 1.0)
            nc.vector.tensor_copy(ones_r[:], ones[:])

            # warm up ncfw before the first real collective. Content is
            # irrelevant; DRAM->DRAM copy avoids any compute dependency, and
            # the trigger sits after the weight loads so it never delays them.
            nc.sync.dma_start(warm_in[:], mask_d.ap()[:, 0, 0:4])
            nc.gpsimd.collective_compute(
                "AllGather",
                mybir.AluOpType.bypass,
                replica_groups=GROUPS,
                ins=[warm_in.opt()],
                outs=[warm_out.opt()],
            )

            late_dmas = {
                0: lambda: nc.sync.dma_start(
                    wq[:], wq_d.ap().rearrange("(a p) n -> p a n", p=P)
                ),
                1: lambda: (
                    nc.sync.dma_start(mask[:], mask_d.ap()),
                    nc.sync.dma_start(
                        wout[:], wout_d.ap().rearrange("(a p) n -> p a n", p=P)
                    ),
                ),
                2: lambda: nc.sync.dma_start(bout[:], bout_d.ap()),
            }

            xss = []

            def load_xs(t):
                xs = wp.tile([P, 8, 512], FP16, name=f"xs{t}", tag="xs", bufs=2)
                xss.append(xs)
                xs_src = xt_d.ap()[:, t * 512 : (t + 1) * 512].rearrange(
                    "(a p) j -> p a j", p=P
                )
                if t == 0:
                    # per-chunk loads so the first matmul starts on chunk 0
                    for c in range(4):
                        nc.sync.dma_start(xs[:, c, :], xs_src[:, c, :])
                else:
                    nc.sync.dma_start(xs[:, 0:4, :], xs_src[:, 0:4, :])
                nc.gpsimd.dma_start(xs[:, 4:8, :], xs_src[:, 4:8, :])

            load_xs(0)
            for t in range(NT):
                xs = xss[t]
                # prefetch next round's x ahead of this round's bounce writes
                # so the sync queue never parks it behind a busy transfer
                if t + 1 < NT:
                    load_xs(t + 1)
                if t in late_dmas:
                    late_dmas[t]()

                kstage = wp.tile([P, 4, 512], FP16, name=f"ks{t}", tag="ks", bufs=2)
                vstage = wp.tile([P, 4, 512], FP16, name=f"vs{t}", tag="vs", bufs=2)
                for dk in range(4):
                    kps = psA.tile([P, 512], F32, name=f"k{t}_{dk}", tag="kv", bufs=8)
                    for c in range(8):
                        nc.tensor.matmul(
                            kps[:], wk[:, c, dk * P : (dk + 1) * P], xs[:, c, :],
                            start=(c == 0), stop=(c == 7),
                        )
                    nc.vector.tensor_copy(kstage[:, dk, :], kps[:])
                for jsub in range(4):
                    vps = psA.tile([P, 512], F32, name=f"v{t}_{jsub}", tag="kv", bufs=8)
                    for c in range(8):
                        nc.tensor.matmul(
                            vps[:], xs[:, c, jsub * P : (jsub + 1) * P], wv[:, c, :],
                            start=(c == 0), stop=(c == 7),
                        )
                    nc.vector.tensor_copy(vstage[:, jsub, :], vps[:])

                nc.sync.dma_start(cc_ins[t][:, 0:2048], kstage[:])
                nc.sync.dma_start(cc_ins[t][:, 2048:4096], vstage[:])
                if t < 3:
                    nc.gpsimd.collective_compute(
                        "AllGather",
                        mybir.AluOpType.bypass,
                        replica_groups=GROUPS,
                        ins=[cc_ins[t].opt()],
                        outs=[cc_outs[t].opt()],
                    )
                for dq in range(4):
                    qps = psA.tile([P, 512], F32, name=f"q{t}_{dq}", tag="kv", bufs=8)
                    for c in range(8):
                        nc.tensor.matmul(
                            qps[:], wq[:, c, dq * P : (dq + 1) * P], xs[:, c, :],
                            start=(c == 0), stop=(c == 7),
                        )
                    nc.vector.tensor_copy(qts[t][:, dq, :], qps[:])

            # read gathered K/V of rounds 0/1 back into rank-major slots;
            # rounds 2/3 are re-read later (interleaved into phase B) to
            # spread readback traffic away from the in-flight collectives.
            def readback(t):
                for r in range(2):
                    s = r * 4 + t
                    nc.sync.dma_start(
                        kt[:, :, s, :],
                        cc_outs[t][r][:, 0:2048].rearrange("p (a j) -> p a j", a=4),
                    )
                    nc.sync.dma_start(
                        vt[:, 4 * s : 4 * s + 4, :],
                        cc_outs[t][r][:, 2048:4096].rearrange("p (a j) -> p a j", a=4),
                    )

            readback(0)
            readback(1)

            psA_cm.__exit__(None, None, None)
            pa.__exit__(None, None, None)

            # ---- Phase B: attention per query super-block u ----
            # The previous block's out-projection is emitted two score-groups
            # into the next block's loop, so the PE never waits on the
            # DVE normalization chain (l -> recip -> attn evict).
            psB_cm = tc.tile_pool(name="psB", bufs=1, space="PSUM")
            psB = psB_cm.__enter__()

            y_dst = y_d.ap().rearrange("(a p) n -> p a n", p=P)
            pending_outproj = None
            pending_finalize = None

            def make_outproj(u, attn):
                def emit():
                    for ic in range(4):
                        y_s = y_ss[(u * 4 + ic) % 4]
                        for doc in range(2):
                            y_ps = psB.tile(
                                [P, 512], F32,
                                name=f"yp{u}_{ic}_{doc}", tag="ka", bufs=3,
                            )
                            for dvc in range(4):
                                nc.tensor.matmul(
                                    y_ps[:],
                                    attn[dvc][:, ic * P : (ic + 1) * P],
                                    wout[:, dvc, doc * 512 : (doc + 1) * 512],
                                    start=(dvc == 0), stop=(dvc == 3),
                                )
                            # eviction adds the bias on the way out (DVE)
                            nc.vector.tensor_add(
                                y_s[:, doc * 512 : (doc + 1) * 512], y_ps[:],
                                bout[:, doc * 512 : (doc + 1) * 512],
                            )
                        # gpsimd queue: the sync queue's pending readbacks must
                        # not delay y writes
                        nc.gpsimd.dma_start(y_dst[:, u * 4 + ic, :], y_s[:])

                return emit

            for u in range(NT):
                qt = qts[u]
                # (slot, kb, off, mask_row); first entry full-width, last
                # entry full-width (carries PSUM start/stop for the PV chain)
                order = []
                for s in range(u):
                    for kb in range(4):
                        order.append((s, kb, 0, None))
                for s in range(4, 4 + u):
                    for kb in range(4):
                        order.append((s, kb, 0, None))
                for kb in range(4):
                    order.append((4 + u, kb, kb * P, 4 + kb))
                for kb in range(4):
                    order.append((u, kb, 0, kb))
                nkb = len(order)

                outT = [
                    psB.tile([P, 512], F32, name=f"o{u}_{d}", tag=f"outT{d}", bufs=1)
                    for d in range(4)
                ]
                p_ts = []

                def attn_v(idx):
                    s, kb, off, _ = order[idx]
                    pt = p_ts[idx]
                    for dvc in range(4):
                        nc.tensor.matmul(
                            outT[dvc][:, off:512],
                            vt[:, 4 * s + kb, dvc * P : (dvc + 1) * P],
                            pt[:, off:512],
                            start=(idx == 0), stop=(idx == nkb - 1),
                        )

                LAG = 3
                for idx, (s, kb, off, mrow) in enumerate(order):
                    s_ps = psB.tile(
                        [P, 512], F32, name=f"s{u}_{idx}", tag="ka", bufs=3
                    )
                    for dkc in range(4):
                        nc.tensor.matmul(
                            s_ps[:, off:512],
                            kt[:, dkc, s, kb * P : (kb + 1) * P],
                            qt[:, dkc, off:512],
                            start=(dkc == 0), stop=(dkc == 3),
                        )
                    pt = pts[idx % 4]
                    p_ts.append(pt)
                    if mrow is not None and off == 0:
                        # halves: DVE mask-mul of half 0 runs under the
                        # ScalarE exp of half 1
                        nc.scalar.activation(pt[:, 0:256], s_ps[:, 0:256],
                                             AF.Exp, scale=SCALE)
                        nc.scalar.activation(pt[:, 256:512], s_ps[:, 256:512],
                                             AF.Exp, scale=SCALE)
                        nc.vector.tensor_mul(pt[:, 0:256], pt[:, 0:256],
                                             mask[:, mrow, 0:256])
                        nc.vector.tensor_mul(pt[:, 256:512], pt[:, 256:512],
                                             mask[:, mrow, 256:512])
                    else:
                        nc.scalar.activation(pt[:, off:512], s_ps[:, off:512],
                                             AF.Exp, scale=SCALE)
                        if mrow is not None:
                            nc.vector.tensor_mul(pt[:, off:512], pt[:, off:512],
                                                 mask[:, mrow, off:512])
                    # last two blocks skip the serial P_acc chain; their sums
                    # ride extra accumulating l-matmuls in finalize instead
                    pacc = paccs[u % 2]
                    if idx == 0:
                        nc.gpsimd.tensor_copy(pacc[:], pt[:])
                    elif idx < nkb - 2:
                        nc.gpsimd.tensor_add(pacc[:, off:512], pacc[:, off:512],
                                             pt[:, off:512])
                    if idx == 0 and pending_finalize is not None:
                        # previous block's l matmul + normalization chain runs
                        # under this block's first score groups
                        pending_finalize()
                        pending_finalize = None
                    if idx >= LAG:
                        attn_v(idx - LAG)
                    if idx == 3 and pending_outproj is not None:
                        pending_outproj()
                        pending_outproj = None
                for idx in range(nkb - LAG, nkb):
                    attn_v(idx)

                if u == 0:
                    # round-3 collective trigger: emitted only now so the
                    # gpsimd queue never parks u0's P_acc chain behind it
                    nc.gpsimd.collective_compute(
                        "AllGather",
                        mybir.AluOpType.bypass,
                        replica_groups=GROUPS,
                        ins=[cc_ins[3].opt()],
                        outs=[cc_outs[3].opt()],
                    )
                    readback(2)
                if u == 1:
                    readback(3)

                attn = [attns[(u % 2) * 4 + d] for d in range(4)]

                def make_finalize(u, outT, attn, pt_tail):
                    def emit():
                        l_ps = psB.tile([P, 512], F32, name=f"l{u}", tag="l",
                                        bufs=1)
                        nc.tensor.matmul(l_ps[:], ones_r[:, 0:P],
                                         paccs[u % 2][:], start=True, stop=False)
                        for i, pt in enumerate(pt_tail):
                            nc.tensor.matmul(l_ps[:], ones[:, 0:P], pt[:],
                                             start=False,
                                             stop=(i == len(pt_tail) - 1))
                        recip = recips[u % 2]
                        nc.vector.reciprocal(recip[:], l_ps[:])
                        for dvc in range(4):
                            nc.vector.tensor_mul(attn[dvc][:], outT[dvc][:],
                                                 recip[:])

                    return emit

                pending_finalize = make_finalize(u, outT, attn, p_ts[nkb - 2 :])
                pending_outproj = make_outproj(u, attn)

            pending_finalize()
            pending_outproj()
            psB_cm.__exit__(None, None, None)

    nc.compile()
    return nc


def _get_nc():
    global _COMPILED
    if _COMPILED is None:
        _COMPILED = _build()
    return _COMPILED


def _make_mask(h: int) -> np.ndarray:
    # rows 0..3: rank0 diagonal strip (global strip 2u vs queries 2u+h)
    # rows 4..7: rank1 diagonal strip (global strip 2u+1 vs queries 2u+h)
    pj = np.arange(P)[:, None, None]
    kb = np.arange(4)[None, :, None]
    il = np.arange(512)[None, None, :]
    m0 = (kb * P + pj - h * 512 - il) <= 0
    m1 = ((1 - h) * 512 + kb * P + pj - il) <= 0
    return np.concatenate([m0, m1], axis=1).astype(np.float16)


def _prep_inputs(x, w_qkv, w_out, b_out):
    wq = np.ascontiguousarray(w_qkv[:, 0:DI]).astype(np.float16)
    wk = np.ascontiguousarray(w_qkv[:, DI : 2 * DI]).astype(np.float16)
    wv = np.ascontiguousarray(w_qkv[:, 2 * DI : 3 * DI]).astype(np.float16)
    wout = np.ascontiguousarray(w_out).astype(np.float16)
    bout = np.broadcast_to(b_out.astype(np.float16), (P, DO)).copy()
    masks = [_make_mask(h) for h in range(2)]
    in_maps = []
    for c in range(NCORES):
        b, h = c // 2, c % 2
        qrows = np.concatenate(
            [np.arange((2 * t + h) * 512, (2 * t + h + 1) * 512) for t in range(NT)]
        )
        xt = np.ascontiguousarray(x[b][qrows].T).astype(np.float16)
        in_maps.append(
            dict(xt=xt, wq=wq, wk=wk, wv=wv, wout=wout, bout=bout, mask=masks[h])
        )
    return in_maps


def _assemble(results):
    out = np.empty((B, N, DO), dtype=np.float32)
    for c in range(NCORES):
        b, h = c // 2, c % 2
        y = results[c]["y"].astype(np.float32)
        for t in range(NT):
            g = 2 * t + h
            out[b, g * 512 : (g + 1) * 512, :] = y[t * 512 : (t + 1) * 512, :]
    return out


def _run(inputs, **kw):
    nc = _get_nc()
    in_maps = _prep_inputs(
        np.asarray(inputs["x"], dtype=np.float32),
        np.asarray(inputs["w_qkv"], dtype=np.float32),
        np.asarray(inputs["w_out"], dtype=np.float32),
        np.asarray(inputs["b_out"], dtype=np.float32),
    )
    res = run_bass_kernel_spmd(nc, in_maps, list(range(NCORES)), **kw)
    return _assemble(res.results), res


def kernel(x, w_qkv, w_out, b_out):
    out, _ = _run(dict(x=x, w_qkv=w_qkv, w_out=w_out, b_out=b_out))
    return out


# revision 39
# speedup vs baseline: 1.0280x; 1.0280x over previous
"""Causal single-head attention (B=4, N=4096, d_in=1024, d_inner=512, d_out=1024)
for 8 Trainium2 NeuronCores.

Sharding: core c -> (batch b=c//2, half h=c%2). Core h of a pair owns the 4
global 512-row blocks {2t+h : t=0..3} of batch b, which serve BOTH as its
query blocks and as the key strips whose K/V projection it computes. The
missing (peer-parity) K/V strips arrive via 4 pipelined pair-wise AllGather
collectives (~1MB contribution each) that overlap with projection compute,
eliminating the duplicated K/V projection of the all-local scheme.

Key-slot layout after the gather is rank-major: kt slot s = r*4 + t holds
global strip 2t+r. Attention per query super-block u processes slots
{0..u} u {4..4+u}; causal masking enters only through a per-core 0/1 mask
input so the instruction stream stays SPMD-uniform. The rank1 diagonal
strip's blocks are truncated via free-dim offsets (queries below the
diagonal are never computed, read, or accumulated).

All matmuls run in fp16 (full PE rate, ~1e-3 rel err overall); PSUM
accumulation is fp32. The softmax denominator is accumulated on the Pool
engine (P_acc += P per block) with a single ones-matmul per super-block,
keeping the PE free of the per-block reduction matmuls.
"""

import sys

if "/opt/trn_rl_repo" not in sys.path:
    sys.path.insert(0, "/opt/trn_rl_repo")

import numpy as np

import concourse.bacc as bacc
import concourse.mybir as mybir
import concourse.tile as tile
from concourse.bass_utils import run_bass_kernel_spmd

P = 128
B, N, DIN, DI, DO = 4, 4096, 1024, 512, 1024
NCORES = 8
NQ = N // 2          # query rows per core (2048)
NT = 4               # rounds / query super-blocks per core
SCALE = float(DI) ** -0.5

F32 = mybir.dt.float32
F32R = mybir.dt.float32r
FP16 = mybir.dt.float16
AF = mybir.ActivationFunctionType

GROUPS = [[0, 1], [2, 3], [4, 5], [6, 7]]

_COMPILED = None


def _build():
    nc = bacc.Bacc(None, target_bir_lowering=False)

    xt_d = nc.dram_tensor("xt", [DIN, NQ], FP16, kind="ExternalInput")
    wq_d = nc.dram_tensor("wq", [DIN, DI], FP16, kind="ExternalInput")
    wk_d = nc.dram_tensor("wk", [DIN, DI], FP16, kind="ExternalInput")
    wv_d = nc.dram_tensor("wv", [DIN, DI], FP16, kind="ExternalInput")
    wout_d = nc.dram_tensor("wout", [DI, DO], FP16, kind="ExternalInput")
    bout_d = nc.dram_tensor("bout", [P, DO], FP16, kind="ExternalInput")
    mask_d = nc.dram_tensor("mask", [P, 8, 512], FP16, kind="ExternalInput")
    y_d = nc.dram_tensor("y", [NQ, DO], FP16, kind="ExternalOutput")

    with tile.TileContext(nc) as tc:
        with tc.tile_pool(name="persist", bufs=1) as pp, tc.tile_pool(
            name="dram", bufs=1, space="DRAM"
        ) as dram:
            kt = pp.tile([P, 4, 8, 512], FP16)     # K^T [dk-chunk, slot, j]
            vt = pp.tile([P, 32, DI], FP16)        # V [kbslot, dv]
            qts = [pp.tile([P, 4, 512], FP16, name=f"qt{u}") for u in range(NT)]
            wout = pp.tile([P, 4, DO], FP16)
            bout = pp.tile([P, DO], FP16)
            mask = pp.tile([P, 8, 512], FP16)
            ones = pp.tile([P, P], FP16)
            ones_r = pp.tile([P, P], F32R)
            # P_acc split across Pool (even blocks) and DVE (odd blocks):
            # two half-length serial chains instead of one, x2 for u parity
            paccs = [pp.tile([P, 512], F32R, name=f"pacc{i}", tag=f"pacc{i}",
                             bufs=1) for i in range(4)]
            # phase-B working tiles live in the persistent pool so the first
            # attention block never waits on phase-A pool teardown
            pts = [pp.tile([P, 512], FP16, name=f"pt{i}", tag=f"pt{i}", bufs=1)
                   for i in range(4)]
            recips = [pp.tile([P, 512], F32, name=f"rec{i}", tag=f"rec{i}", bufs=1)
                      for i in range(2)]
            attns = [pp.tile([P, 512], FP16, name=f"at{i}", tag=f"at{i}", bufs=1)
                     for i in range(8)]
            y_ss = [pp.tile([P, DO], FP16, name=f"ysb{i}", tag=f"ysb{i}", bufs=1)
                    for i in range(4)]

            cc_ins = [dram.tile([P, 4096], FP16, name=f"ccin{t}") for t in range(NT)]
            cc_outs = [
                dram.tile([2, P, 4096], FP16, name=f"ccout{t}") for t in range(NT)
            ]
            warm_in = dram.tile([P, 4], FP16)
            warm_out = dram.tile([2, P, 4], FP16)

            # ---- Phase A: project own-strip K/V/Q; pair-gather K/V ----
            pa = tc.tile_pool(name="phaseA", bufs=1)
            wp = pa.__enter__()
            psA_cm = tc.tile_pool(name="psA", bufs=1, space="PSUM")
            psA = psA_cm.__enter__()

            # per-chunk K-weight tiles: the first matmul's dependency is one
            # 128x512 DMA, not the whole 1MB load
            wks = [wp.tile([P, DI], FP16, name=f"wk{c}", tag=f"wk{c}")
                   for c in range(8)]
            wv = wp.tile([P, 8, DI], FP16)
            wq = wp.tile([P, 8, DI], FP16)

            wk_src = wk_d.ap().rearrange("(a p) n -> p a n", p=P)
            wv_src = wv_d.ap().rearrange("(a p) n -> p a n", p=P)
            for c in range(8):
                nc.gpsimd.dma_start(wks[c][:], wk_src[:, c, :])
            for c in range(8):
                nc.gpsimd.dma_start(wv[:, c, :], wv_src[:, c, :])
            nc.vector.memset(ones[:], 1.0)
            nc.vector.tensor_copy(ones_r[:], ones[:])

            # warm up ncfw before the first real collective. Content is
            # irrelevant; DRAM->DRAM copy avoids any compute dependency, and
            # the trigger sits after the weight loads so it never delays them.
            nc.sync.dma_start(warm_in[:], mask_d.ap()[:, 0, 0:4])
            nc.gpsimd.collective_compute(
                "AllGather",
                mybir.AluOpType.bypass,
                replica_groups=GROUPS,
                ins=[warm_in.opt()],
                outs=[warm_out.opt()],
            )

            # wq gates round 0's Q projection -> earliest; mask/wout/bout are
            # phase-B-only -> latest, off the bandwidth-critical early window
            late_dmas = {
                2: lambda: (
                    nc.sync.dma_start(mask[:], mask_d.ap()),
                    nc.sync.dma_start(
                        wout[:], wout_d.ap().rearrange("(a p) n -> p a n", p=P)
                    ),
                ),
                3: lambda: nc.sync.dma_start(bout[:], bout_d.ap()),
            }

            xss = []

            def load_xs(t):
                xs = wp.tile([P, 8, 512], FP16, name=f"xs{t}", tag="xs", bufs=2)
                xss.append(xs)
                xs_src = xt_d.ap()[:, t * 512 : (t + 1) * 512].rearrange(
                    "(a p) j -> p a j", p=P
                )
                nc.sync.dma_start(xs[:, 0:4, :], xs_src[:, 0:4, :])
                nc.gpsimd.dma_start(xs[:, 4:8, :], xs_src[:, 4:8, :])

            load_xs(0)
            nc.sync.dma_start(wq[:], wq_d.ap().rearrange("(a p) n -> p a n", p=P))
            for t in range(NT):
                xs = xss[t]
                # prefetch next round's x ahead of this round's bounce writes
                # so the sync queue never parks it behind a busy transfer
                if t + 1 < NT:
                    load_xs(t + 1)
                if t in late_dmas:
                    late_dmas[t]()

                kstage = wp.tile([P, 4, 512], FP16, name=f"ks{t}", tag="ks", bufs=2)
                vstage = wp.tile([P, 4, 512], FP16, name=f"vs{t}", tag="vs", bufs=2)
                for dk in range(4):
                    kps = psA.tile([P, 512], F32, name=f"k{t}_{dk}", tag="kv", bufs=8)
                    for c in range(8):
                        nc.tensor.matmul(
                            kps[:], wks[c][:, dk * P : (dk + 1) * P], xs[:, c, :],
                            start=(c == 0), stop=(c == 7),
                        )
                    nc.vector.tensor_copy(kstage[:, dk, :], kps[:])
                for jsub in range(4):
                    vps = psA.tile([P, 512], F32, name=f"v{t}_{jsub}", tag="kv", bufs=8)
                    for c in range(8):
                        nc.tensor.matmul(
                            vps[:], xs[:, c, jsub * P : (jsub + 1) * P], wv[:, c, :],
                            start=(c == 0), stop=(c == 7),
                        )
                    nc.vector.tensor_copy(vstage[:, jsub, :], vps[:])

                nc.sync.dma_start(cc_ins[t][:, 0:2048], kstage[:])
                nc.sync.dma_start(cc_ins[t][:, 2048:4096], vstage[:])
                if t < 3:
                    nc.gpsimd.collective_compute(
                        "AllGather",
                        mybir.AluOpType.bypass,
                        replica_groups=GROUPS,
                        ins=[cc_ins[t].opt()],
                        outs=[cc_outs[t].opt()],
                    )
                for dq in range(4):
                    qps = psA.tile([P, 512], F32, name=f"q{t}_{dq}", tag="kv", bufs=8)
                    for c in range(8):
                        nc.tensor.matmul(
                            qps[:], wq[:, c, dq * P : (dq + 1) * P], xs[:, c, :],
                            start=(c == 0), stop=(c == 7),
                        )
                    nc.vector.tensor_copy(qts[t][:, dq, :], qps[:])

            # read gathered K/V of rounds 0/1 back into rank-major slots;
            # rounds 2/3 are re-read later (interleaved into phase B) to
            # spread readback traffic away from the in-flight collectives.
            def readback(t):
                for r in range(2):
                    s = r * 4 + t
                    nc.sync.dma_start(
                        kt[:, :, s, :],
                        cc_outs[t][r][:, 0:2048].rearrange("p (a j) -> p a j", a=4),
                    )
                    nc.sync.dma_start(
                        vt[:, 4 * s : 4 * s + 4, :],
                        cc_outs[t][r][:, 2048:4096].rearrange("p (a j) -> p a j", a=4),
                    )

            readback(0)
            readback(1)

            psA_cm.__exit__(None, None, None)
            pa.__exit__(None, None, None)

            # ---- Phase B: attention per query super-block u ----
            # The previous block's out-projection is emitted two score-groups
            # into the next block's loop, so the PE never waits on the
            # DVE normalization chain (l -> recip -> attn evict).
            psB_cm = tc.tile_pool(name="psB", bufs=1, space="PSUM")
            psB = psB_cm.__enter__()

            y_dst = y_d.ap().rearrange("(a p) n -> p a n", p=P)
            pending_outproj = None
            pending_finalize = None

            def make_outproj(u, attn):
                def emit():
                    for ic in range(4):
                        y_s = y_ss[(u * 4 + ic) % 4]
                        for doc in range(2):
                            y_ps = psB.tile(
                                [P, 512], F32,
                                name=f"yp{u}_{ic}_{doc}", tag="ka", bufs=3,
                            )
                            for dvc in range(4):
                                nc.tensor.matmul(
                                    y_ps[:],
                                    attn[dvc][:, ic * P : (ic + 1) * P],
                                    wout[:, dvc, doc * 512 : (doc + 1) * 512],
                                    start=(dvc == 0), stop=(dvc == 3),
                                )
                            # eviction adds the bias on the way out (DVE)
                            nc.vector.tensor_add(
                                y_s[:, doc * 512 : (doc + 1) * 512], y_ps[:],
                                bout[:, doc * 512 : (doc + 1) * 512],
                            )
                        # gpsimd queue: the sync queue's pending readbacks must
                        # not delay y writes
                        nc.gpsimd.dma_start(y_dst[:, u * 4 + ic, :], y_s[:])

                return emit

            for u in range(NT):
                qt = qts[u]
                # (slot, kb, off, mask_row); first entry full-width, last
                # entry full-width (carries PSUM start/stop for the PV chain)
                order = []
                for s in range(u):
                    for kb in range(4):
                        order.append((s, kb, 0, None))
                for s in range(4, 4 + u):
                    for kb in range(4):
                        order.append((s, kb, 0, None))
                for kb in range(4):
                    order.append((4 + u, kb, kb * P, 4 + kb))
                for kb in range(4):
                    order.append((u, kb, 0, kb))
                nkb = len(order)

                outT = [
                    psB.tile([P, 512], F32, name=f"o{u}_{d}", tag=f"outT{d}", bufs=1)
                    for d in range(4)
                ]
                p_ts = []

                def attn_v(idx):
                    s, kb, off, _ = order[idx]
                    pt = p_ts[idx]
                    for dvc in range(4):
                        nc.tensor.matmul(
                            outT[dvc][:, off:512],
                            vt[:, 4 * s + kb, dvc * P : (dvc + 1) * P],
                            pt[:, off:512],
                            start=(idx == 0), stop=(idx == nkb - 1),
                        )

                LAG = 3
                for idx, (s, kb, off, mrow) in enumerate(order):
                    s_ps = psB.tile(
                        [P, 512], F32, name=f"s{u}_{idx}", tag="ka", bufs=3
                    )
                    for dkc in range(4):
                        nc.tensor.matmul(
                            s_ps[:, off:512],
                            kt[:, dkc, s, kb * P : (kb + 1) * P],
                            qt[:, dkc, off:512],
                            start=(dkc == 0), stop=(dkc == 3),
                        )
                    pt = pts[idx % 4]
                    p_ts.append(pt)
                    if mrow is not None and off == 0:
                        # halves: DVE mask-mul of half 0 runs under the
                        # ScalarE exp of half 1
                        nc.scalar.activation(pt[:, 0:256], s_ps[:, 0:256],
                                             AF.Exp, scale=SCALE)
                        nc.scalar.activation(pt[:, 256:512], s_ps[:, 256:512],
                                             AF.Exp, scale=SCALE)
                        nc.vector.tensor_mul(pt[:, 0:256], pt[:, 0:256],
                                             mask[:, mrow, 0:256])
                        nc.vector.tensor_mul(pt[:, 256:512], pt[:, 256:512],
                                             mask[:, mrow, 256:512])
                    else:
                        nc.scalar.activation(pt[:, off:512], s_ps[:, off:512],
                                             AF.Exp, scale=SCALE)
                        if mrow is not None:
                            nc.vector.tensor_mul(pt[:, off:512], pt[:, off:512],
                                                 mask[:, mrow, off:512])
                    # last two blocks skip the serial P_acc chains; their sums
                    # ride extra accumulating l-matmuls in finalize instead.
                    # u=0 keeps a single Pool chain (its first blocks are
                    # offset-truncated, so a second chain can't start clean).
                    split = u > 0
                    lane = idx % 2 if split else 0
                    eng = nc.vector if lane else nc.gpsimd
                    pacc = paccs[(u % 2) * 2 + lane]
                    if idx < (2 if split else 1):
                        eng.tensor_copy(pacc[:], pt[:])
                    elif idx < nkb - 2:
                        eng.tensor_add(pacc[:, off:512], pacc[:, off:512],
                                       pt[:, off:512])
                    if idx == 0 and pending_finalize is not None:
                        # previous block's l matmul + normalization chain runs
                        # under this block's first score groups
                        pending_finalize()
                        pending_finalize = None
                    if idx >= LAG:
                        attn_v(idx - LAG)
                    if idx == 3 and pending_outproj is not None:
                        pending_outproj()
                        pending_outproj = None
                for idx in range(nkb - LAG, nkb):
                    attn_v(idx)

                if u == 0:
                    # round-3 collective trigger: emitted only now so the
                    # gpsimd queue never parks u0's P_acc chain behind it
                    nc.gpsimd.collective_compute(
                        "AllGather",
                        mybir.AluOpType.bypass,
                        replica_groups=GROUPS,
                        ins=[cc_ins[3].opt()],
                        outs=[cc_outs[3].opt()],
                    )
                    readback(2)
                if u == 1:
                    readback(3)

                attn = [attns[(u % 2) * 4 + d] for d in range(4)]

                def make_finalize(u, outT, attn, pt_tail):
                    def emit():
                        l_ps = psB.tile([P, 512], F32, name=f"l{u}", tag="l",
                                        bufs=1)
                        nc.tensor.matmul(l_ps[:], ones_r[:, 0:P],
                                         paccs[(u % 2) * 2][:],
                                         start=True, stop=False)
                        if u > 0:
                            nc.tensor.matmul(l_ps[:], ones_r[:, 0:P],
                                             paccs[(u % 2) * 2 + 1][:],
                                             start=False, stop=False)
                        for i, pt in enumerate(pt_tail):
                            nc.tensor.matmul(l_ps[:], ones[:, 0:P], pt[:],
                                             start=False,
                                             stop=(i == len(pt_tail) - 1))
                        recip = recips[u % 2]
                        nc.vector.reciprocal(recip[:], l_ps[:])
                        for dvc in range(4):
                            nc.vector.tensor_mul(attn[dvc][:], outT[dvc][:],
                                                 recip[:])

                    return emit

                pending_finalize = make_finalize(u, outT, attn, p_ts[nkb - 2 :])
                pending_outproj = make_outproj(u, attn)

            pending_finalize()
            pending_outproj()
            psB_cm.__exit__(None, None, None)

    nc.compile()
    return nc


def _get_nc():
    global _COMPILED
    if _COMPILED is None:
        _COMPILED = _build()
    return _COMPILED


def _make_mask(h: int) -> np.ndarray:
    # rows 0..3: rank0 diagonal strip (global strip 2u vs queries 2u+h)
    # rows 4..7: rank1 diagonal strip (global strip 2u+1 vs queries 2u+h)
    pj = np.arange(P)[:, None, None]
    kb = np.arange(4)[None, :, None]
    il = np.arange(512)[None, None, :]
    m0 = (kb * P + pj - h * 512 - il) <= 0
    m1 = ((1 - h) * 512 + kb * P + pj - il) <= 0
    return np.concatenate([m0, m1], axis=1).astype(np.float16)


def _prep_inputs(x, w_qkv, w_out, b_out):
    wq = np.ascontiguousarray(w_qkv[:, 0:DI]).astype(np.float16)
    wk = np.ascontiguousarray(w_qkv[:, DI : 2 * DI]).astype(np.float16)
    wv = np.ascontiguousarray(w_qkv[:, 2 * DI : 3 * DI]).astype(np.float16)
    wout = np.ascontiguousarray(w_out).astype(np.float16)
    bout = np.broadcast_to(b_out.astype(np.float16), (P, DO)).copy()
    masks = [_make_mask(h) for h in range(2)]
    in_maps = []
    for c in range(NCORES):
        b, h = c // 2, c % 2
        qrows = np.concatenate(
            [np.arange((2 * t + h) * 512, (2 * t + h + 1) * 512) for t in range(NT)]
        )
        xt = np.ascontiguousarray(x[b][qrows].T).astype(np.float16)
        in_maps.append(
            dict(xt=xt, wq=wq, wk=wk, wv=wv, wout=wout, bout=bout, mask=masks[h])
        )
    return in_maps


def _assemble(results):
    out = np.empty((B, N, DO), dtype=np.float32)
    for c in range(NCORES):
        b, h = c // 2, c % 2
        y = results[c]["y"].astype(np.float32)
        for t in range(NT):
            g = 2 * t + h
            out[b, g * 512 : (g + 1) * 512, :] = y[t * 512 : (t + 1) * 512, :]
    return out


def _run(inputs, **kw):
    nc = _get_nc()
    in_maps = _prep_inputs(
        np.asarray(inputs["x"], dtype=np.float32),
        np.asarray(inputs["w_qkv"], dtype=np.float32),
        np.asarray(inputs["w_out"], dtype=np.float32),
        np.asarray(inputs["b_out"], dtype=np.float32),
    )
    res = run_bass_kernel_spmd(nc, in_maps, list(range(NCORES)), **kw)
    return _assemble(res.results), res


def kernel(x, w_qkv, w_out, b_out):
    out, _ = _run(dict(x=x, w_qkv=w_qkv, w_out=w_out, b_out=b_out))
    return out
